# revision 29
# baseline (speedup 1.0000x reference)
"""Trainium2 Bass kernel for DLRA conv layer (3x3 low-rank conv + bias + relu).

Computes: relu(extract_patches_3x3(x) @ U @ W2 + bias) for the step-selected
factor set (W2 = S @ V folded on host for step 2). Sharded over H across 8
NeuronCores (28 rows each, 1-px halo resolved on host).

Device dataflow per core, per image:
  stage 1 (576->100): per quad (2 row-pairs A/B = 4 output rows, 896 px):
    - 3 full-array K=128 matmuls per row-pair contract shift pairs
      (di,0)+(di,1) via bufA = [x ; x<<1col] (2 HBM loads, no SBUF shift).
    - the 3 leftover dj=2 singles are K=64 and run as CONCURRENT row-tiled
      matmul pairs (tile_position (0,0)/(64,0)): the top half-array computes
      shift (s,2) for row-pair A from bufA's top (x, cols 2:), the bottom
      half-array computes it for row-pair B from bufA's bottom (x<<1col,
      cols 1:). Each pair costs ~one matmul slot -> 9 slots per 896 px
      (4.5/row-pair vs 6 in the naive packing).
    - ACT drains PSUM -> z1 (fp16, [100 x 6272] per image).
  stage 2 (100->256): w2 filter-halves are the STATIONARY operand (2 weight
    loads per image instead of 49); z1 streams as N=448 moving chunks into
    2-bank PSUM tiles [128 x 1024]. Output is [filters, pixels].
  epilogue: DVE adds the bias (8/14 tiles straight from PSUM at 1x, 6/14 via
    an ACT fp32->fp16 copy then DVE add at 2x SBUF mode), DVE relu at 4x fp16
    mode per filter-half, fp16 store (halves HBM write traffic). Host
    transposes [fh,f,px] -> (H,W,256) and casts fp32.

  A burst of dummy matmuls at t=0 warms the PE HAM clock-gate (2.4 GHz needs
  ~3.4us of sustained PE activity) while the first image loads.
"""

import numpy as np
from contextlib import ExitStack

import concourse.bacc as bacc
import concourse.tile as tile
import concourse.mybir as mybir
from concourse.bass_utils import run_bass_kernel_spmd

B, H, W, C = 8, 224, 224, 64
KH = KW = 3
RANK = 100
FILTERS = 256
IN_DIM = KH * KW * C  # 576

NCORES = 8
HS = H // NCORES          # 28 output rows per core
HSH = HS + 2              # input rows incl halo
WP = W + 2                # padded width
XL = HSH * WP             # flat image-strip length per channel (6780)
NPIX = HS * W             # 6272 pixels per image strip
RPP = 2 * W               # 448 px per row-pair (stage-1 matmul N)
NQ = HS // 4              # 7 quads (2 row-pairs each) per image
NT2 = 14                  # stage-2 tiles per image (2 fh x 7 blocks of 896px)
MP = 128                  # padded stationary columns (rank 100 -> 128)

# stage-2 tiles whose PSUM drain goes via an ACT copy + 2x DVE add (the rest
# are direct 1x DVE adds from PSUM) -- balances DVE vs ACT load.
VIA_ACT = frozenset({1, 3, 5, 7, 9, 11})
N_WARMUP_MM = 44

F32 = mybir.dt.float32
MM_DT = mybir.dt.float16
MM_NP = np.float16

_CACHE = {}


def _build_nc():
    nc = bacc.Bacc("TRN2", target_bir_lowering=False, debug=False,
                   num_devices=NCORES)
    xt = nc.dram_tensor("xt", [B, C, XL], MM_DT, kind="ExternalInput").ap()
    w1 = nc.dram_tensor("w1", [2 * C, 6 * MP], MM_DT,
                        kind="ExternalInput").ap()
    w2 = nc.dram_tensor("w2", [RANK, FILTERS], MM_DT,
                        kind="ExternalInput").ap()
    bias = nc.dram_tensor("bias", [MP, 2 * NPIX], MM_DT,
                          kind="ExternalInput").ap()
    out = nc.dram_tensor("out", [B, 2, MP, NPIX], MM_DT,
                         kind="ExternalOutput").ap()
    fcopy = mybir.ActivationFunctionType.Copy
    add = mybir.AluOpType.add

    with tile.TileContext(nc) as tc, ExitStack() as ctx:
        const = ctx.enter_context(tc.tile_pool(name="const", bufs=1))
        xpool = ctx.enter_context(tc.tile_pool(name="xpool", bufs=2))
        z1pool = ctx.enter_context(tc.tile_pool(name="z1pool", bufs=2))
        ps1pool = ctx.enter_context(
            tc.tile_pool(name="ps1", bufs=2, space="PSUM"))
        ps2pool = ctx.enter_context(
            tc.tile_pool(name="ps2", bufs=2, space="PSUM"))
        ogpool = ctx.enter_context(tc.tile_pool(name="ogpool", bufs=4))
        t2pool = ctx.enter_context(tc.tile_pool(name="t2pool", bufs=3))

        # consts: w1/w2 on the scalar DMA queue (small, land first so the
        # warm-up matmuls can start ~immediately), bias follows there too.
        w1_t = const.tile([2 * C, 6 * MP], MM_DT, name="w1_t")
        nc.scalar.dma_start(w1_t[:], w1[:])
        w2_t = const.tile([RANK, FILTERS], MM_DT, name="w2_t")
        nc.scalar.dma_start(w2_t[:], w2[:])
        bias_t = const.tile([MP, 2 * NPIX], MM_DT, name="bias_t")

        # HAM warm-up: keep the PE busy from ~t=0 so the clock-gate opens
        # (K=8/8) before the first real matmul. Operands come from a locally
        # memset tile so the burst does not wait on any DMA.
        dummy = const.tile([MP, 512], MM_DT, name="dummy")
        nc.vector.memset(dummy[:], 0.0)
        for _ in range(N_WARMUP_MM):
            psd = ps1pool.tile([MP, 1024], F32, name="psd", tag="psAB")
            nc.tensor.matmul(psd[:, 0:512], lhsT=dummy[:, 0:MP],
                             rhs=dummy[:],
                             start=True, stop=True, skip_group_check=True)

        def load_image(img):
            """bufA = [x ; x shifted 1 col] via two HBM loads (no serialized
            SBUF->SBUF hop). Tail element of the bottom half stays garbage:
            it is never read (reads stop at flat index XL-2)."""
            bufa = xpool.tile([2 * C, XL], MM_DT, name="bufa", tag="bufa")
            nc.sync.dma_start(bufa[0:C, :], xt[img])
            nc.sync.dma_start(bufa[C:2 * C, 0:XL - 1], xt[img, :, 1:XL])
            return bufa

        def stage1_quad(bufa, q, z1t):
            """Conv 576->100 for quad q (row-pairs 2q, 2q+1; 896 px)."""
            av = bufa[:, 0:XL].rearrange("c (r w) -> c r w", w=WP)
            rA = 4 * q          # padded-row base of row-pair A
            rB = 4 * q + 2
            # one 2-bank PSUM tile for both row-pairs so the z1 drain is a
            # single strided ACT copy (halves the ACT op count)
            psAB = ps1pool.tile([MP, 1024], F32, name="psAB", tag="psAB")
            psA = psAB[:, 0:RPP]
            psB = psAB[:, 512:512 + RPP]
            for p in range(3):  # shift pairs (p,0)+(p,1), K=128
                lw = w1_t[:, p * MP:(p + 1) * MP]
                nc.tensor.matmul(psA, lhsT=lw,
                                 rhs=av[:, rA + p:rA + p + 2, 0:W],
                                 start=(p == 0), stop=False,
                                 skip_group_check=True)
                nc.tensor.matmul(psB, lhsT=lw,
                                 rhs=av[:, rB + p:rB + p + 2, 0:W],
                                 start=(p == 0), stop=False,
                                 skip_group_check=True)
            for s in range(3):  # singles (s,2), K=64, concurrent row-tiles
                sl = (3 + s) * MP
                last = (s == 2)
                nc.tensor.matmul(psA, lhsT=w1_t[0:C, sl:sl + MP],
                                 rhs=av[0:C, rA + s:rA + s + 2, 2:2 + W],
                                 start=False, stop=last,
                                 skip_group_check=True, tile_position=(0, 0))
                nc.tensor.matmul(psB, lhsT=w1_t[C:2 * C, sl:sl + MP],
                                 rhs=av[C:2 * C, rB + s:rB + s + 2, 1:1 + W],
                                 start=False, stop=last,
                                 skip_group_check=True, tile_position=(C, 0))
            zsrc = psAB[0:RANK, :].rearrange("r (a b) -> r a b", b=512)
            nc.scalar.activation(
                z1t[:, 2 * q * RPP:(2 * q + 2) * RPP].rearrange(
                    "r (a b) -> r a b", b=RPP),
                zsrc[:, :, 0:RPP], fcopy)

        ogs = {}  # (img, fh) -> og tile

        def stage2_tile(img, fh, blk, z1t, via_act, blk_relu=False):
            """100->256 for one (fh, 896px block); bias add into og."""
            base = blk * 2 * RPP
            if (img, fh) not in ogs:
                ogs[(img, fh)] = ogpool.tile([MP, NPIX], MM_DT, name="og",
                                             tag="og")
            og = ogs[(img, fh)]
            lw2 = w2_t[:, fh * MP:(fh + 1) * MP]
            ps2 = ps2pool.tile([MP, 1024], F32, name="ps2", tag="ps2")
            nc.tensor.matmul(ps2[:, 0:448], lhsT=lw2,
                             rhs=z1t[:, base:base + 448],
                             start=True, stop=True, skip_group_check=True)
            nc.tensor.matmul(ps2[:, 512:960], lhsT=lw2,
                             rhs=z1t[:, base + 448:base + 896],
                             start=True, stop=True, skip_group_check=True)
            src = ps2[:].rearrange("f (a b) -> f a b", b=512)[:, :, 0:448]
            bc = bias_t[:, fh * NPIX + base:fh * NPIX + base + 896]
            oc = og[:, base:base + 896]
            if via_act:
                t2 = t2pool.tile([MP, 896], MM_DT, name="t2", tag="t2")
                t2v = t2[:].rearrange("f (a b) -> f a b", b=448)
                nc.scalar.activation(t2v, src, fcopy)
                nc.vector.tensor_tensor(oc, t2[:], bc, add)
            else:
                bcv = bc.rearrange("f (a b) -> f a b", b=448)
                ocv = oc.rearrange("f (a b) -> f a b", b=448)
                nc.vector.tensor_tensor(ocv, src, bcv, add)
            if blk_relu:  # incremental relu (last image: shortens the tail)
                nc.vector.tensor_scalar_max(oc, oc, 0.0)
                if blk == 3:  # first column-half final: store it early
                    nc.gpsimd.dma_start(out[img, fh, 0:C, 0:3584],
                                        og[0:C, 0:3584])
                    nc.sync.dma_start(out[img, fh, C:MP, 0:3584],
                                      og[C:MP, 0:3584])
            if blk == NQ - 1:  # filter-half complete: relu + store, retire
                if not blk_relu:
                    nc.vector.tensor_scalar_max(og[:], og[:], 0.0)
                    lo = 0
                else:
                    lo = 3584
                # store halves ride both free DMA queues to halve latency;
                # split on the partition dim to keep DMA packets large.
                nc.gpsimd.dma_start(out[img, fh, 0:C, lo:NPIX],
                                    og[0:C, lo:NPIX])
                nc.sync.dma_start(out[img, fh, C:MP, lo:NPIX],
                                  og[C:MP, lo:NPIX])
                del ogs[(img, fh)]

        # Stage-2 schedule: both filter-halves of image i trail its stage-1 by
        # one quad (the per-quad w2 stationary alternation is hidden by the
        # LDWEIGHTS pull-ahead). Alternate via_act to balance DVE vs ACT.
        # Image 0 trails by TWO quads so its first bias add lands after the
        # bias DMA (which queues behind x0 on the sync queue).
        bufa_cur = load_image(0)
        nc.sync.dma_start(bias_t[:], bias[:])
        for img in range(B):
            bufa_next = load_image(img + 1) if img + 1 < B else None
            z1t = z1pool.tile([RANK, NPIX], MM_DT, name="z1", tag="z1")
            last = (img == B - 1)
            # lag 2 decouples the s2 matmuls from ACT z1-copy jitter; the
            # last image runs lag 1 + incremental relu to shorten the tail.
            lag = 1 if last else 2
            for q in range(NQ):
                stage1_quad(bufa_cur, q, z1t)
                if q >= lag:
                    stage2_tile(img, 0, q - lag, z1t, via_act=(q % 2 == 1),
                                blk_relu=last)
                    stage2_tile(img, 1, q - lag, z1t, via_act=(q % 2 == 0),
                                blk_relu=last)
            for blk in range(NQ - lag, NQ):
                stage2_tile(img, 0, blk, z1t, via_act=False, blk_relu=last)
                stage2_tile(img, 1, blk, z1t, via_act=not last, blk_relu=last)
            bufa_cur = bufa_next

    nc.compile()
    return nc


def _get_nc():
    if "nc" not in _CACHE:
        _CACHE["nc"] = _build_nc()
    return _CACHE["nc"]


def _prep_inputs(x, k, l_t, s, aux_U, aux_Unp1, aux_Vt, aux_Vtnp1, b, aux_b,
                 step):
    step = int(np.asarray(step))
    x = np.ascontiguousarray(np.asarray(x, dtype=np.float32))
    if step == 0:
        U, W2, bias = np.asarray(k), np.asarray(aux_Vt), np.asarray(aux_b)
    elif step == 1:
        U, W2, bias = np.asarray(aux_U), np.asarray(l_t), np.asarray(aux_b)
    else:
        U = np.asarray(aux_Unp1)
        W2 = (np.asarray(s, np.float64) @ np.asarray(aux_Vtnp1, np.float64))
        bias = np.asarray(b)
    U = U.astype(np.float32)
    W2 = np.ascontiguousarray(W2.astype(MM_NP))
    bias = np.asarray(bias, np.float32)

    # channel-major, zero-padded H and W, fp16
    xpad = np.zeros((B, H + 2, W + 2, C), np.float32)
    xpad[:, 1:-1, 1:-1, :] = x
    xpad_t = np.ascontiguousarray(xpad.transpose(0, 3, 1, 2)).astype(MM_NP)

    # stage-1 stationary slots [128, 6*128]:
    #   p=0..2: top=blocks[p,0], bottom=blocks[p,1] (pairs, K=128)
    #   p=3..5: blocks[p-3,2] duplicated into both halves (concurrent K=64
    #           row-tiles for row-pairs A and B)
    blocks = U.reshape(KH, KW, C, RANK)
    w1p = np.zeros((6, 2 * C, MP), np.float32)
    for p in range(3):
        w1p[p, 0:C, 0:RANK] = blocks[p, 0]
        w1p[p, C:2 * C, 0:RANK] = blocks[p, 1]
    for s_ in range(3):
        w1p[3 + s_, 0:C, 0:RANK] = blocks[s_, 2]
        w1p[3 + s_, C:2 * C, 0:RANK] = blocks[s_, 2]
    w1 = np.ascontiguousarray(
        w1p.transpose(1, 0, 2).reshape(2 * C, 6 * MP)).astype(MM_NP)

    in_maps = []
    for i in range(NCORES):
        xt_i = np.ascontiguousarray(
            xpad_t[:, :, HS * i:HS * i + HSH, :]).reshape(B, C, XL)
        # bias strip -> [f, fh*NPIX + px] (transposed, filter-major)
        bs = bias[HS * i:HS * (i + 1)].reshape(NPIX, FILTERS)
        bt = np.ascontiguousarray(bs.T).astype(MM_NP)      # (256, NPIX)
        b_i = np.ascontiguousarray(
            np.concatenate([bt[0:MP], bt[MP:FILTERS]], axis=1))
        in_maps.append({"xt": xt_i, "w1": w1, "w2": W2, "bias": b_i})
    return in_maps


def _assemble(results):
    strips = [
        results[i]["out"].transpose(0, 3, 1, 2).reshape(B, HS, W, FILTERS)
        for i in range(NCORES)
    ]
    return np.concatenate(strips, axis=1).astype(np.float32)


def run(trace=False, **inputs):
    in_maps = _prep_inputs(**inputs)
    nc = _get_nc()
    res = run_bass_kernel_spmd(nc, in_maps, list(range(NCORES)), trace=trace)
    return _assemble(res.results), res


def kernel(**inputs):
    out, _ = run(trace=False, **inputs)
    return out


# revision 30
# speedup vs baseline: 1.0893x; 1.0893x over previous
"""Trainium2 Bass kernel for DLRA conv layer (3x3 low-rank conv + bias + relu).

Computes: relu(extract_patches_3x3(x) @ U @ W2 + bias) for the step-selected
factor set (W2 = S @ V folded on host for step 2). Sharded over H across 8
NeuronCores (28 rows each, 1-px halo resolved on host).

Device dataflow per core, per image:
  stage 1 (576->100): per quad (2 row-pairs A/B = 4 output rows, 896 px):
    - 3 full-array K=128 matmuls per row-pair contract shift pairs
      (di,0)+(di,1) via bufA = [x ; x<<1col] (2 HBM loads, no SBUF shift).
    - the 3 leftover dj=2 singles are K=64 and run as CONCURRENT row-tiled
      matmul pairs (tile_position (0,0)/(64,0)): the top half-array computes
      shift (s,2) for row-pair A from bufA's top (x, cols 2:), the bottom
      half-array computes it for row-pair B from bufA's bottom (x<<1col,
      cols 1:). Each pair costs ~one matmul slot -> 9 slots per 896 px
      (4.5/row-pair vs 6 in the naive packing).
    - ACT drains PSUM -> z1 (fp16, [100 x 6272] per image).
  stage 2 (100->256): w2 filter-halves are the STATIONARY operand (2 weight
    loads per image instead of 49); z1 streams as N=448 moving chunks into
    2-bank PSUM tiles [128 x 1024]. Output is [filters, pixels].
  epilogue: DVE adds the bias (8/14 tiles straight from PSUM at 1x, 6/14 via
    an ACT fp32->fp16 copy then DVE add at 2x SBUF mode), DVE relu at 4x fp16
    mode per filter-half, fp16 store (halves HBM write traffic). Host
    transposes [fh,f,px] -> (H,W,256) and casts fp32.

  A burst of dummy matmuls at t=0 warms the PE HAM clock-gate (2.4 GHz needs
  ~3.4us of sustained PE activity) while the first image loads.
"""

import numpy as np
from contextlib import ExitStack

import concourse.bacc as bacc
import concourse.tile as tile
import concourse.mybir as mybir
from concourse.bass_utils import run_bass_kernel_spmd

B, H, W, C = 8, 224, 224, 64
KH = KW = 3
RANK = 100
FILTERS = 256
IN_DIM = KH * KW * C  # 576

NCORES = 8
HS = H // NCORES          # 28 output rows per core
HSH = HS + 2              # input rows incl halo
WP = W + 2                # padded width
XL = HSH * WP             # flat image-strip length per channel (6780)
NPIX = HS * W             # 6272 pixels per image strip
RPP = 2 * W               # 448 px per row-pair (stage-1 matmul N)
NQ = HS // 4              # 7 quads (2 row-pairs each) per image
NT2 = 14                  # stage-2 tiles per image (2 fh x 7 blocks of 896px)
MP = 128                  # padded stationary columns (rank 100 -> 128)

N_WARMUP_MM = 44  # ~9.5us of dummy matmuls: opens the HAM clock-gate while x0 loads

F32 = mybir.dt.float32
MM_DT = mybir.dt.float16
MM_NP = np.float16

_CACHE = {}


def _build_nc():
    nc = bacc.Bacc("TRN2", target_bir_lowering=False, debug=False,
                   num_devices=NCORES)
    xt = nc.dram_tensor("xt", [B, C, XL], MM_DT, kind="ExternalInput").ap()
    w1 = nc.dram_tensor("w1", [2 * C, 6 * MP], MM_DT,
                        kind="ExternalInput").ap()
    w2 = nc.dram_tensor("w2", [RANK, FILTERS], MM_DT,
                        kind="ExternalInput").ap()
    bias = nc.dram_tensor("bias", [MP, 2 * NPIX], MM_DT,
                          kind="ExternalInput").ap()
    out = nc.dram_tensor("out", [B, 2, MP, NPIX], MM_DT,
                         kind="ExternalOutput").ap()
    fcopy = mybir.ActivationFunctionType.Copy
    add = mybir.AluOpType.add

    with tile.TileContext(nc) as tc, ExitStack() as ctx:
        const = ctx.enter_context(tc.tile_pool(name="const", bufs=1))
        xpool = ctx.enter_context(tc.tile_pool(name="xpool", bufs=2))
        z1pool = ctx.enter_context(tc.tile_pool(name="z1pool", bufs=2))
        ps1pool = ctx.enter_context(
            tc.tile_pool(name="ps1", bufs=2, space="PSUM"))
        ps2pool = ctx.enter_context(
            tc.tile_pool(name="ps2", bufs=2, space="PSUM"))
        ogpool = ctx.enter_context(tc.tile_pool(name="ogpool", bufs=4))
        t2pool = ctx.enter_context(tc.tile_pool(name="t2pool", bufs=3))

        # consts: w1/w2 on the scalar DMA queue (small, land first so the
        # warm-up matmuls can start ~immediately), bias follows there too.
        w1_t = const.tile([2 * C, 6 * MP], MM_DT, name="w1_t")
        nc.scalar.dma_start(w1_t[:], w1[:])
        w2_t = const.tile([RANK, FILTERS], MM_DT, name="w2_t")
        nc.scalar.dma_start(w2_t[:], w2[:])
        bias_t = const.tile([MP, 2 * NPIX], MM_DT, name="bias_t")

        # HAM warm-up: keep the PE busy from ~t=0 so the clock-gate opens
        # (K=8/8) before the first real matmul. Operands come from a locally
        # memset tile so the burst does not wait on any DMA.
        dummy = const.tile([MP, 512], MM_DT, name="dummy")
        nc.vector.memset(dummy[:], 0.0)
        for _ in range(N_WARMUP_MM):
            psd = ps1pool.tile([MP, 1024], F32, name="psd", tag="psAB")
            nc.tensor.matmul(psd[:, 0:512], lhsT=dummy[:, 0:MP],
                             rhs=dummy[:],
                             start=True, stop=True, skip_group_check=True)

        def load_image(img):
            """bufA = [x ; x shifted 1 col] via two HBM loads (no serialized
            SBUF->SBUF hop). Tail element of the bottom half stays garbage:
            it is never read (reads stop at flat index XL-2)."""
            bufa = xpool.tile([2 * C, XL], MM_DT, name="bufa", tag="bufa")
            nc.sync.dma_start(bufa[0:C, :], xt[img])
            nc.sync.dma_start(bufa[C:2 * C, 0:XL - 1], xt[img, :, 1:XL])
            return bufa

        def stage1_quad(bufa, q, z1t):
            """Conv 576->100 for quad q (row-pairs 2q, 2q+1; 896 px)."""
            av = bufa[:, 0:XL].rearrange("c (r w) -> c r w", w=WP)
            rA = 4 * q          # padded-row base of row-pair A
            rB = 4 * q + 2
            # one 2-bank PSUM tile for both row-pairs so the z1 drain is a
            # single strided ACT copy (halves the ACT op count)
            psAB = ps1pool.tile([MP, 1024], F32, name="psAB", tag="psAB")
            psA = psAB[:, 0:RPP]
            psB = psAB[:, 512:512 + RPP]
            for p in range(3):  # shift pairs (p,0)+(p,1), K=128
                lw = w1_t[:, p * MP:(p + 1) * MP]
                nc.tensor.matmul(psA, lhsT=lw,
                                 rhs=av[:, rA + p:rA + p + 2, 0:W],
                                 start=(p == 0), stop=False,
                                 skip_group_check=True)
                nc.tensor.matmul(psB, lhsT=lw,
                                 rhs=av[:, rB + p:rB + p + 2, 0:W],
                                 start=(p == 0), stop=False,
                                 skip_group_check=True)
            for s in range(3):  # singles (s,2), K=64, concurrent row-tiles
                sl = (3 + s) * MP
                last = (s == 2)
                nc.tensor.matmul(psA, lhsT=w1_t[0:C, sl:sl + MP],
                                 rhs=av[0:C, rA + s:rA + s + 2, 2:2 + W],
                                 start=False, stop=last,
                                 skip_group_check=True, tile_position=(0, 0))
                nc.tensor.matmul(psB, lhsT=w1_t[C:2 * C, sl:sl + MP],
                                 rhs=av[C:2 * C, rB + s:rB + s + 2, 1:1 + W],
                                 start=False, stop=last,
                                 skip_group_check=True, tile_position=(C, 0))
            zsrc = psAB[0:RANK, :].rearrange("r (a b) -> r a b", b=512)
            nc.scalar.activation(
                z1t[:, 2 * q * RPP:(2 * q + 2) * RPP].rearrange(
                    "r (a b) -> r a b", b=RPP),
                zsrc[:, :, 0:RPP], fcopy)

        ogs = {}  # (img, fh) -> og tile

        def stage2_tile(img, fh, blk, z1t, via_act, blk_relu=False):
            """100->256 for one (fh, 896px block); bias add into og."""
            base = blk * 2 * RPP
            if (img, fh) not in ogs:
                ogs[(img, fh)] = ogpool.tile([MP, NPIX], MM_DT, name="og",
                                             tag="og")
            og = ogs[(img, fh)]
            lw2 = w2_t[:, fh * MP:(fh + 1) * MP]
            ps2 = ps2pool.tile([MP, 1024], F32, name="ps2", tag="ps2")
            nc.tensor.matmul(ps2[:, 0:448], lhsT=lw2,
                             rhs=z1t[:, base:base + 448],
                             start=True, stop=True, skip_group_check=True)
            nc.tensor.matmul(ps2[:, 512:960], lhsT=lw2,
                             rhs=z1t[:, base + 448:base + 896],
                             start=True, stop=True, skip_group_check=True)
            src = ps2[:].rearrange("f (a b) -> f a b", b=512)[:, :, 0:448]
            bc = bias_t[:, fh * NPIX + base:fh * NPIX + base + 896]
            oc = og[:, base:base + 896]
            if via_act:
                t2 = t2pool.tile([MP, 896], MM_DT, name="t2", tag="t2")
                t2v = t2[:].rearrange("f (a b) -> f a b", b=448)
                nc.scalar.activation(t2v, src, fcopy)
                nc.vector.tensor_tensor(oc, t2[:], bc, add)
            else:
                bcv = bc.rearrange("f (a b) -> f a b", b=448)
                ocv = oc.rearrange("f (a b) -> f a b", b=448)
                nc.vector.tensor_tensor(ocv, src, bcv, add)
            if blk_relu:  # incremental relu (last image: shortens the tail)
                nc.vector.tensor_scalar_max(oc, oc, 0.0)
                if blk == 3:  # first column-half final: store it early
                    nc.gpsimd.dma_start(out[img, fh, 0:C, 0:3584],
                                        og[0:C, 0:3584])
                    nc.sync.dma_start(out[img, fh, C:MP, 0:3584],
                                      og[C:MP, 0:3584])
            if blk == NQ - 1:  # filter-half complete: relu + store, retire
                if not blk_relu:
                    nc.vector.tensor_scalar_max(og[:], og[:], 0.0)
                    lo = 0
                else:
                    lo = 3584
                # store halves ride both free DMA queues to halve latency;
                # split on the partition dim to keep DMA packets large.
                nc.gpsimd.dma_start(out[img, fh, 0:C, lo:NPIX],
                                    og[0:C, lo:NPIX])
                nc.sync.dma_start(out[img, fh, C:MP, lo:NPIX],
                                  og[C:MP, lo:NPIX])
                del ogs[(img, fh)]

        # Stage-2 schedule: both filter-halves of image i trail its stage-1 by
        # one quad (the per-quad w2 stationary alternation is hidden by the
        # LDWEIGHTS pull-ahead). Alternate via_act to balance DVE vs ACT.
        # Image 0 trails by TWO quads so its first bias add lands after the
        # bias DMA (which queues behind x0 on the sync queue).
        bufa_cur = load_image(0)
        nc.sync.dma_start(bias_t[:], bias[:])
        for img in range(B):
            bufa_next = load_image(img + 1) if img + 1 < B else None
            z1t = z1pool.tile([RANK, NPIX], MM_DT, name="z1", tag="z1")
            last = (img == B - 1)
            # lag 2 decouples the s2 matmuls from ACT z1-copy jitter; the
            # last image runs lag 1 + incremental relu to shorten the tail.
            lag = 1 if last else 2
            for q in range(NQ):
                stage1_quad(bufa_cur, q, z1t)
                if q >= lag:
                    stage2_tile(img, 0, q - lag, z1t, via_act=(q % 2 == 1),
                                blk_relu=last)
                    stage2_tile(img, 1, q - lag, z1t, via_act=(q % 2 == 0),
                                blk_relu=last)
            for blk in range(NQ - lag, NQ):
                stage2_tile(img, 0, blk, z1t, via_act=False, blk_relu=last)
                stage2_tile(img, 1, blk, z1t, via_act=not last, blk_relu=last)
            bufa_cur = bufa_next

    nc.compile()
    return nc


def _get_nc():
    if "nc" not in _CACHE:
        _CACHE["nc"] = _build_nc()
    return _CACHE["nc"]


def _prep_inputs(x, k, l_t, s, aux_U, aux_Unp1, aux_Vt, aux_Vtnp1, b, aux_b,
                 step):
    step = int(np.asarray(step))
    x = np.ascontiguousarray(np.asarray(x, dtype=np.float32))
    if step == 0:
        U, W2, bias = np.asarray(k), np.asarray(aux_Vt), np.asarray(aux_b)
    elif step == 1:
        U, W2, bias = np.asarray(aux_U), np.asarray(l_t), np.asarray(aux_b)
    else:
        U = np.asarray(aux_Unp1)
        W2 = (np.asarray(s, np.float64) @ np.asarray(aux_Vtnp1, np.float64))
        bias = np.asarray(b)
    U = U.astype(np.float32)
    W2 = np.ascontiguousarray(W2.astype(MM_NP))
    bias = np.asarray(bias, np.float32)

    # channel-major, zero-padded H and W, fp16
    xpad = np.zeros((B, H + 2, W + 2, C), np.float32)
    xpad[:, 1:-1, 1:-1, :] = x
    xpad_t = np.ascontiguousarray(xpad.transpose(0, 3, 1, 2)).astype(MM_NP)

    # stage-1 stationary slots [128, 6*128]:
    #   p=0..2: top=blocks[p,0], bottom=blocks[p,1] (pairs, K=128)
    #   p=3..5: blocks[p-3,2] duplicated into both halves (concurrent K=64
    #           row-tiles for row-pairs A and B)
    blocks = U.reshape(KH, KW, C, RANK)
    w1p = np.zeros((6, 2 * C, MP), np.float32)
    for p in range(3):
        w1p[p, 0:C, 0:RANK] = blocks[p, 0]
        w1p[p, C:2 * C, 0:RANK] = blocks[p, 1]
    for s_ in range(3):
        w1p[3 + s_, 0:C, 0:RANK] = blocks[s_, 2]
        w1p[3 + s_, C:2 * C, 0:RANK] = blocks[s_, 2]
    w1 = np.ascontiguousarray(
        w1p.transpose(1, 0, 2).reshape(2 * C, 6 * MP)).astype(MM_NP)

    in_maps = []
    for i in range(NCORES):
        xt_i = np.ascontiguousarray(
            xpad_t[:, :, HS * i:HS * i + HSH, :]).reshape(B, C, XL)
        # bias strip -> [f, fh*NPIX + px] (transposed, filter-major)
        bs = bias[HS * i:HS * (i + 1)].reshape(NPIX, FILTERS)
        bt = np.ascontiguousarray(bs.T).astype(MM_NP)      # (256, NPIX)
        b_i = np.ascontiguousarray(
            np.concatenate([bt[0:MP], bt[MP:FILTERS]], axis=1))
        in_maps.append({"xt": xt_i, "w1": w1, "w2": W2, "bias": b_i})
    return in_maps


def _assemble(results):
    strips = [
        results[i]["out"].transpose(0, 3, 1, 2).reshape(B, HS, W, FILTERS)
        for i in range(NCORES)
    ]
    return np.concatenate(strips, axis=1).astype(np.float32)


def run(trace=False, **inputs):
    in_maps = _prep_inputs(**inputs)
    nc = _get_nc()
    res = run_bass_kernel_spmd(nc, in_maps, list(range(NCORES)), trace=trace)
    return _assemble(res.results), res


def kernel(**inputs):
    out, _ = run(trace=False, **inputs)
    return out


# revision 31
# speedup vs baseline: 1.1002x; 1.0100x over previous
"""Trainium2 Bass kernel for DLRA conv layer (3x3 low-rank conv + bias + relu).

Computes: relu(extract_patches_3x3(x) @ U @ W2 + bias) for the step-selected
factor set (W2 = S @ V folded on host for step 2). Sharded over H across 8
NeuronCores (28 rows each, 1-px halo resolved on host).

Device dataflow per core, per image:
  stage 1 (576->100): per quad (2 row-pairs A/B = 4 output rows, 896 px):
    - 3 full-array K=128 matmuls per row-pair contract shift pairs
      (di,0)+(di,1) via bufA = [x ; x<<1col] (2 HBM loads, no SBUF shift).
    - the 3 leftover dj=2 singles are K=64 and run as CONCURRENT row-tiled
      matmul pairs (tile_position (0,0)/(64,0)): the top half-array computes
      shift (s,2) for row-pair A from bufA's top (x, cols 2:), the bottom
      half-array computes it for row-pair B from bufA's bottom (x<<1col,
      cols 1:). Each pair costs ~one matmul slot -> 9 slots per 896 px
      (4.5/row-pair vs 6 in the naive packing).
    - ACT drains PSUM -> z1 (fp16, [100 x 6272] per image).
  stage 2 (100->256): w2 filter-halves are the STATIONARY operand (2 weight
    loads per image instead of 49); z1 streams as N=448 moving chunks into
    2-bank PSUM tiles [128 x 1024]. Output is [filters, pixels].
  epilogue: DVE adds the bias (half the tiles straight from PSUM at 1x, half
    via an ACT fp32->fp16 copy then DVE add at 2x SBUF mode -- balances the
    two engines), DVE relu at 4x fp16 mode, fp16 store (halves HBM write
    traffic) split across the gpsimd+sync DMA queues on the partition dim.
    Host transposes [fh,f,px] -> (H,W,256) and casts fp32.

  Scheduling: stage-2 trails stage-1 by 2 quads (1 on the last image, which
  also relus incrementally and stores its first column-half early, to
  shorten the drain tail). A burst of dummy matmuls at t=0 warms the PE HAM
  clock-gate (2.4 GHz needs ~3.4us of sustained PE activity) while the first
  image loads; the bias DMA rides the sync queue between image 0 and image 1
  so it never competes with x0 for DMA engines.
"""

import numpy as np
from contextlib import ExitStack

import concourse.bacc as bacc
import concourse.tile as tile
import concourse.mybir as mybir
from concourse.bass_utils import run_bass_kernel_spmd

B, H, W, C = 8, 224, 224, 64
KH = KW = 3
RANK = 100
FILTERS = 256
IN_DIM = KH * KW * C  # 576

NCORES = 8
HS = H // NCORES          # 28 output rows per core
HSH = HS + 2              # input rows incl halo
WP = W + 2                # padded width
XL = HSH * WP             # flat image-strip length per channel (6780)
NPIX = HS * W             # 6272 pixels per image strip
RPP = 2 * W               # 448 px per row-pair (stage-1 matmul N)
NQ = HS // 4              # 7 quads (2 row-pairs each) per image
NT2 = 14                  # stage-2 tiles per image (2 fh x 7 blocks of 896px)
MP = 128                  # padded stationary columns (rank 100 -> 128)

N_WARMUP_MM = 44  # ~9.5us of dummy matmuls: opens the HAM clock-gate while x0 loads

F32 = mybir.dt.float32
MM_DT = mybir.dt.float16
MM_NP = np.float16

_CACHE = {}


def _build_nc():
    nc = bacc.Bacc("TRN2", target_bir_lowering=False, debug=False,
                   num_devices=NCORES)
    xt = nc.dram_tensor("xt", [B, C, XL], MM_DT, kind="ExternalInput").ap()
    w1 = nc.dram_tensor("w1", [2 * C, 6 * MP], MM_DT,
                        kind="ExternalInput").ap()
    w2 = nc.dram_tensor("w2", [RANK, FILTERS], MM_DT,
                        kind="ExternalInput").ap()
    bias = nc.dram_tensor("bias", [MP, 2 * NPIX], MM_DT,
                          kind="ExternalInput").ap()
    out = nc.dram_tensor("out", [B, 2, MP, NPIX], MM_DT,
                         kind="ExternalOutput").ap()
    fcopy = mybir.ActivationFunctionType.Copy
    add = mybir.AluOpType.add

    with tile.TileContext(nc) as tc, ExitStack() as ctx:
        const = ctx.enter_context(tc.tile_pool(name="const", bufs=1))
        xpool = ctx.enter_context(tc.tile_pool(name="xpool", bufs=2))
        z1pool = ctx.enter_context(tc.tile_pool(name="z1pool", bufs=2))
        ps1pool = ctx.enter_context(
            tc.tile_pool(name="ps1", bufs=2, space="PSUM"))
        ps2pool = ctx.enter_context(
            tc.tile_pool(name="ps2", bufs=2, space="PSUM"))
        ogpool = ctx.enter_context(tc.tile_pool(name="ogpool", bufs=4))
        t2pool = ctx.enter_context(tc.tile_pool(name="t2pool", bufs=3))

        # consts: w1/w2 on the scalar DMA queue (small, land first so the
        # warm-up matmuls can start ~immediately), bias follows there too.
        w1_t = const.tile([2 * C, 6 * MP], MM_DT, name="w1_t")
        nc.scalar.dma_start(w1_t[:], w1[:])
        w2_t = const.tile([RANK, FILTERS], MM_DT, name="w2_t")
        nc.scalar.dma_start(w2_t[:], w2[:])
        bias_t = const.tile([MP, 2 * NPIX], MM_DT, name="bias_t")

        # HAM warm-up: keep the PE busy from ~t=0 so the clock-gate opens
        # (K=8/8) before the first real matmul. Operands come from a locally
        # memset tile so the burst does not wait on any DMA.
        dummy = const.tile([MP, 512], MM_DT, name="dummy")
        nc.vector.memset(dummy[:], 0.0)
        for _ in range(N_WARMUP_MM):
            psd = ps1pool.tile([MP, 1024], F32, name="psd", tag="psAB")
            nc.tensor.matmul(psd[:, 0:512], lhsT=dummy[:, 0:MP],
                             rhs=dummy[:],
                             start=True, stop=True, skip_group_check=True)

        def load_image(img):
            """bufA = [x ; x shifted 1 col] via two HBM loads (no serialized
            SBUF->SBUF hop). Tail element of the bottom half stays garbage:
            it is never read (reads stop at flat index XL-2)."""
            bufa = xpool.tile([2 * C, XL], MM_DT, name="bufa", tag="bufa")
            nc.sync.dma_start(bufa[0:C, :], xt[img])
            nc.sync.dma_start(bufa[C:2 * C, 0:XL - 1], xt[img, :, 1:XL])
            return bufa

        def stage1_quad(bufa, q, z1t):
            """Conv 576->100 for quad q (row-pairs 2q, 2q+1; 896 px)."""
            av = bufa[:, 0:XL].rearrange("c (r w) -> c r w", w=WP)
            rA = 4 * q          # padded-row base of row-pair A
            rB = 4 * q + 2
            # one 2-bank PSUM tile for both row-pairs so the z1 drain is a
            # single strided ACT copy (halves the ACT op count)
            psAB = ps1pool.tile([MP, 1024], F32, name="psAB", tag="psAB")
            psA = psAB[:, 0:RPP]
            psB = psAB[:, 512:512 + RPP]
            for p in range(3):  # shift pairs (p,0)+(p,1), K=128
                lw = w1_t[:, p * MP:(p + 1) * MP]
                nc.tensor.matmul(psA, lhsT=lw,
                                 rhs=av[:, rA + p:rA + p + 2, 0:W],
                                 start=(p == 0), stop=False,
                                 skip_group_check=True)
                nc.tensor.matmul(psB, lhsT=lw,
                                 rhs=av[:, rB + p:rB + p + 2, 0:W],
                                 start=(p == 0), stop=False,
                                 skip_group_check=True)
            for s in range(3):  # singles (s,2), K=64, concurrent row-tiles
                sl = (3 + s) * MP
                last = (s == 2)
                nc.tensor.matmul(psA, lhsT=w1_t[0:C, sl:sl + MP],
                                 rhs=av[0:C, rA + s:rA + s + 2, 2:2 + W],
                                 start=False, stop=last,
                                 skip_group_check=True, tile_position=(0, 0))
                nc.tensor.matmul(psB, lhsT=w1_t[C:2 * C, sl:sl + MP],
                                 rhs=av[C:2 * C, rB + s:rB + s + 2, 1:1 + W],
                                 start=False, stop=last,
                                 skip_group_check=True, tile_position=(C, 0))
            zsrc = psAB[0:RANK, :].rearrange("r (a b) -> r a b", b=512)
            nc.scalar.activation(
                z1t[:, 2 * q * RPP:(2 * q + 2) * RPP].rearrange(
                    "r (a b) -> r a b", b=RPP),
                zsrc[:, :, 0:RPP], fcopy)

        ogs = {}  # (img, fh) -> og tile

        def stage2_tile(img, fh, blk, z1t, via_act, blk_relu=False):
            """100->256 for one (fh, 896px block); bias add into og."""
            base = blk * 2 * RPP
            if (img, fh) not in ogs:
                ogs[(img, fh)] = ogpool.tile([MP, NPIX], MM_DT, name="og",
                                             tag="og")
            og = ogs[(img, fh)]
            lw2 = w2_t[:, fh * MP:(fh + 1) * MP]
            ps2 = ps2pool.tile([MP, 1024], F32, name="ps2", tag="ps2")
            nc.tensor.matmul(ps2[:, 0:448], lhsT=lw2,
                             rhs=z1t[:, base:base + 448],
                             start=True, stop=True, skip_group_check=True)
            nc.tensor.matmul(ps2[:, 512:960], lhsT=lw2,
                             rhs=z1t[:, base + 448:base + 896],
                             start=True, stop=True, skip_group_check=True)
            src = ps2[:].rearrange("f (a b) -> f a b", b=512)[:, :, 0:448]
            bc = bias_t[:, fh * NPIX + base:fh * NPIX + base + 896]
            oc = og[:, base:base + 896]
            if via_act:
                t2 = t2pool.tile([MP, 896], MM_DT, name="t2", tag="t2")
                t2v = t2[:].rearrange("f (a b) -> f a b", b=448)
                nc.scalar.activation(t2v, src, fcopy)
                nc.vector.tensor_tensor(oc, t2[:], bc, add)
            else:
                bcv = bc.rearrange("f (a b) -> f a b", b=448)
                ocv = oc.rearrange("f (a b) -> f a b", b=448)
                nc.vector.tensor_tensor(ocv, src, bcv, add)
            if blk_relu:  # incremental relu (last image: shortens the tail)
                nc.vector.tensor_scalar_max(oc, oc, 0.0)
                if blk == 3:  # first column-half final: store it early
                    nc.gpsimd.dma_start(out[img, fh, 0:C, 0:3584],
                                        og[0:C, 0:3584])
                    nc.sync.dma_start(out[img, fh, C:MP, 0:3584],
                                      og[C:MP, 0:3584])
            if blk == NQ - 1:  # filter-half complete: relu + store, retire
                if not blk_relu:
                    nc.vector.tensor_scalar_max(og[:], og[:], 0.0)
                    lo = 0
                else:
                    lo = 3584
                # store halves ride both free DMA queues to halve latency;
                # split on the partition dim to keep DMA packets large.
                nc.gpsimd.dma_start(out[img, fh, 0:C, lo:NPIX],
                                    og[0:C, lo:NPIX])
                nc.sync.dma_start(out[img, fh, C:MP, lo:NPIX],
                                  og[C:MP, lo:NPIX])
                del ogs[(img, fh)]

        # Stage-2 schedule: both filter-halves of image i trail its stage-1 by
        # one quad (the per-quad w2 stationary alternation is hidden by the
        # LDWEIGHTS pull-ahead). Alternate via_act to balance DVE vs ACT.
        # Image 0 trails by TWO quads so its first bias add lands after the
        # bias DMA (which queues behind x0 on the sync queue).
        bufa_cur = load_image(0)
        nc.sync.dma_start(bias_t[:], bias[:])
        for img in range(B):
            bufa_next = load_image(img + 1) if img + 1 < B else None
            z1t = z1pool.tile([RANK, NPIX], MM_DT, name="z1", tag="z1")
            last = (img == B - 1)
            # lag 2 decouples the s2 matmuls from ACT z1-copy jitter; the
            # last image runs lag 1 + incremental relu to shorten the tail.
            lag = 1 if last else 2
            for q in range(NQ):
                stage1_quad(bufa_cur, q, z1t)
                if q >= lag:
                    stage2_tile(img, 0, q - lag, z1t, via_act=(q % 2 == 1),
                                blk_relu=last)
                    stage2_tile(img, 1, q - lag, z1t, via_act=(q % 2 == 0),
                                blk_relu=last)
            for blk in range(NQ - lag, NQ):
                stage2_tile(img, 0, blk, z1t, via_act=False, blk_relu=last)
                stage2_tile(img, 1, blk, z1t, via_act=not last, blk_relu=last)
            bufa_cur = bufa_next

    nc.compile()
    return nc


def _get_nc():
    if "nc" not in _CACHE:
        _CACHE["nc"] = _build_nc()
    return _CACHE["nc"]


def _prep_inputs(x, k, l_t, s, aux_U, aux_Unp1, aux_Vt, aux_Vtnp1, b, aux_b,
                 step):
    step = int(np.asarray(step))
    x = np.ascontiguousarray(np.asarray(x, dtype=np.float32))
    if step == 0:
        U, W2, bias = np.asarray(k), np.asarray(aux_Vt), np.asarray(aux_b)
    elif step == 1:
        U, W2, bias = np.asarray(aux_U), np.asarray(l_t), np.asarray(aux_b)
    else:
        U = np.asarray(aux_Unp1)
        W2 = (np.asarray(s, np.float64) @ np.asarray(aux_Vtnp1, np.float64))
        bias = np.asarray(b)
    U = U.astype(np.float32)
    W2 = np.ascontiguousarray(W2.astype(MM_NP))
    bias = np.asarray(bias, np.float32)

    # channel-major, zero-padded H and W, fp16
    xpad = np.zeros((B, H + 2, W + 2, C), np.float32)
    xpad[:, 1:-1, 1:-1, :] = x
    xpad_t = np.ascontiguousarray(xpad.transpose(0, 3, 1, 2)).astype(MM_NP)

    # stage-1 stationary slots [128, 6*128]:
    #   p=0..2: top=blocks[p,0], bottom=blocks[p,1] (pairs, K=128)
    #   p=3..5: blocks[p-3,2] duplicated into both halves (concurrent K=64
    #           row-tiles for row-pairs A and B)
    blocks = U.reshape(KH, KW, C, RANK)
    w1p = np.zeros((6, 2 * C, MP), np.float32)
    for p in range(3):
        w1p[p, 0:C, 0:RANK] = blocks[p, 0]
        w1p[p, C:2 * C, 0:RANK] = blocks[p, 1]
    for s_ in range(3):
        w1p[3 + s_, 0:C, 0:RANK] = blocks[s_, 2]
        w1p[3 + s_, C:2 * C, 0:RANK] = blocks[s_, 2]
    w1 = np.ascontiguousarray(
        w1p.transpose(1, 0, 2).reshape(2 * C, 6 * MP)).astype(MM_NP)

    in_maps = []
    for i in range(NCORES):
        xt_i = np.ascontiguousarray(
            xpad_t[:, :, HS * i:HS * i + HSH, :]).reshape(B, C, XL)
        # bias strip -> [f, fh*NPIX + px] (transposed, filter-major)
        bs = bias[HS * i:HS * (i + 1)].reshape(NPIX, FILTERS)
        bt = np.ascontiguousarray(bs.T).astype(MM_NP)      # (256, NPIX)
        b_i = np.ascontiguousarray(
            np.concatenate([bt[0:MP], bt[MP:FILTERS]], axis=1))
        in_maps.append({"xt": xt_i, "w1": w1, "w2": W2, "bias": b_i})
    return in_maps


def _assemble(results):
    strips = [
        results[i]["out"].transpose(0, 3, 1, 2).reshape(B, HS, W, FILTERS)
        for i in range(NCORES)
    ]
    return np.concatenate(strips, axis=1).astype(np.float32)


def run(trace=False, **inputs):
    in_maps = _prep_inputs(**inputs)
    nc = _get_nc()
    res = run_bass_kernel_spmd(nc, in_maps, list(range(NCORES)), trace=trace)
    return _assemble(res.results), res


def kernel(**inputs):
    out, _ = run(trace=False, **inputs)
    return out
